# revision 1
# baseline (speedup 1.0000x reference)
"""DHHGCN Trainium2 kernel: 8-core sample-pruned 2-layer GNN.

Sharding (per the dest-partition hint): each of the 8 cores owns 1/8 of the
sampled (user, item) pairs.  The item-side 2-layer computation is pruned to
the rows reachable from the core's samples (samples + 1-hop in-neighbours,
R ~= 8.6K of 100K).  Host prep = sharding: destination-sorted padded
column-major CSR shards (128-slot tiles) with the per-edge source rows of the
*static* tables (emb/pri/cat) shipped as bf16 message shards; conv rows for
the core's user samples sliced + transposed; edge values shipped separately.
Device does all FLOPs: val-multiply (DVE), segment_sum (TensorE accumulate,
feature-major), dense base/gate/linear matmuls, tanh/relu (ACT), the dense
user-conv matmul, layer-2 gathers from the device-computed e1' table
(4-queue SWDGE dma_gather), and the final score reduction.
"""
import numpy as np
import ml_dtypes

P = 128
D = 64
NCORES = 8


# ----------------------------------------------------------------- host prep

def _round_up(x, m):
    return ((x + m - 1) // m) * m


def _build_csr(n_dest_pad, dst_pos, src, val):
    """Column-major padded CSR (layout B). Returns (tiles [(t,k)...] t-major,
    idx [ntiles*128] int64 src ids, val [ntiles*128] f32)."""
    if len(dst_pos) == 0:
        return [], np.zeros(0, np.int64), np.zeros(0, np.float32)
    order = np.argsort(dst_pos, kind="stable")
    ds, ss, vs = dst_pos[order], src[order], val[order]
    counts = np.bincount(ds, minlength=n_dest_pad)
    starts = np.concatenate([[0], np.cumsum(counts)])
    kidx = np.arange(len(ds)) - starts[ds]
    tk = np.stack([ds // P, kidx], 1)
    uniq = np.unique(tk, axis=0)
    uo = np.lexsort((uniq[:, 1], uniq[:, 0]))
    tiles = [tuple(x) for x in uniq[uo]]
    tile_pos = {t: j for j, t in enumerate(tiles)}
    idx = np.zeros(len(tiles) * P, np.int64)
    vals = np.zeros(len(tiles) * P, np.float32)
    slot = np.array([tile_pos[(d // P, k)] for d, k in zip(ds, kidx)],
                    np.int64) * P + (ds % P)
    idx[slot] = ss
    vals[slot] = vs
    return tiles, idx, vals


def _edges_of(rows, grouped):
    starts, src_s, val_s = grouped
    rows = np.asarray(rows, np.int64)
    cnt = (starts[rows + 1] - starts[rows]).astype(np.int64)
    total = int(cnt.sum())
    if total == 0:
        return (np.zeros(0, np.int64), np.zeros(0, np.int64),
                np.zeros(0, np.float32))
    dsts = np.repeat(np.arange(len(rows), dtype=np.int64), cnt)
    pos = np.concatenate([np.arange(starts[r], starts[r] + c)
                          for r, c in zip(rows, cnt) if c]) if total else None
    return dsts, src_s[pos], val_s[pos]


def _group_edges(idx2, val, emb_n):
    dst, src = idx2[0].astype(np.int64), idx2[1].astype(np.int64)
    o = np.argsort(dst, kind="stable")
    dst_s, src_s, val_s = dst[o], src[o], val[o].astype(np.float32)
    starts = np.concatenate([[0], np.cumsum(np.bincount(dst_s, minlength=emb_n))])
    return starts, src_s, val_s


def _csr_union_layout(all_tiles, T):
    Kt = np.zeros(T, np.int64)
    for tiles in all_tiles:
        if tiles:
            cnt = np.bincount([t for t, k in tiles], minlength=T)
            Kt = np.maximum(Kt, cnt)
    Kt[0] = max(Kt[0], 1)            # ensure >= 1 tile overall
    layout = [(t, k) for t in range(T) for k in range(int(Kt[t]))]
    return layout, {tk: j for j, tk in enumerate(layout)}


def _pack(tiles, idx, val, lpos, ntl):
    IDX = np.zeros(ntl * P, np.int64)
    VAL = np.zeros(ntl * P, np.float32)
    for j, tk in enumerate(tiles):
        jj = lpos[tk]
        IDX[jj * P:(jj + 1) * P] = idx[j * P:(j + 1) * P]
        VAL[jj * P:(jj + 1) * P] = val[j * P:(j + 1) * P]
    return IDX, VAL


def _wrap16(a):
    """dma_gather index wrap: [N] -> [128, N/16]; flat i = s*16 + c."""
    return np.tile(np.ascontiguousarray(a.astype(np.int16).reshape(-1, 16).T),
                   (8, 1))


def prepare_inputs(inputs):
    meta = {}
    in_maps = [{} for _ in range(NCORES)]
    emb_n = inputs["A_emb"].shape[0]
    UA = inputs["user_a"].shape[0]
    BATCH = inputs["item_sample_a"].shape[0]
    NSLOT = BATCH // NCORES
    assert NSLOT % P == 0
    meta["NSLOT"] = NSLOT
    meta["UA"] = UA
    meta["BATCH"] = BATCH

    doms = [("a", "A", "conv_au", "user_a", "user_sample_a", "item_sample_a"),
            ("b", "B", "conv_bu", "user_b", "user_sample_b", "item_sample_b")]

    plans = {}
    for dl, DU, convk, userk, usk, isk in doms:
        adjg = _group_edges(inputs[f"{DU}_adj_idx"], inputs[f"{DU}_adj_val"], emb_n)
        vpg = _group_edges(inputs[f"{DU}_vp_idx"], inputs[f"{DU}_vp_val"], emb_n)
        vcg = _group_edges(inputs[f"{DU}_vc_idx"], inputs[f"{DU}_vc_val"], emb_n)
        adeg = (adjg[0][1:] - adjg[0][:-1])
        item_sample = inputs[isk].astype(np.int64)
        for core in range(NCORES):
            s = item_sample[core * NSLOT:(core + 1) * NSLOT]
            pi2 = np.argsort(-adeg[s], kind="stable")
            s_sorted = s[pi2]
            e2_dst, e2_src, e2_val = _edges_of(s_sorted, adjg)
            extra = np.setdiff1d(np.unique(e2_src), np.unique(s_sorted))
            R_orig = np.concatenate([s_sorted, extra])
            pi1 = np.argsort(-adeg[R_orig], kind="stable")
            R_orig = R_orig[pi1]
            R = len(R_orig)
            pos_of = np.full(emb_n, -1, np.int64)
            pos_of[R_orig[::-1]] = np.arange(R - 1, -1, -1)
            plans[(dl, core)] = dict(
                pi2=pi2, s_sorted=s_sorted, R_orig=R_orig, R=R,
                slot_pos=pos_of[s_sorted],
                l1=_edges_of(R_orig, adjg),
                vp1=_edges_of(R_orig, vpg),
                vc1=_edges_of(R_orig, vcg),
                e2=(e2_dst, pos_of[e2_src], e2_val),
                vp2=_edges_of(s_sorted, vpg),
                vc2=_edges_of(s_sorted, vcg),
            )

    R_pad = _round_up(max(pl["R"] for pl in plans.values()), P)
    T1 = R_pad // P
    meta["R_pad"] = R_pad

    for dl, DU, convk, userk, usk, isk in doms:
        emb = np.asarray(inputs[f"{DU}_emb"], np.float32)
        pri = np.asarray(inputs[f"{DU}_pri"], np.float32)
        cat = np.asarray(inputs[f"{DU}_cat"], np.float32)
        conv = np.asarray(inputs[convk], np.float32)
        user_samp = inputs[usk].astype(np.int64)

        built = {c: {} for c in range(NCORES)}
        for core in range(NCORES):
            pl = plans[(dl, core)]
            built[core]["l1"] = _build_csr(R_pad, *pl["l1"])
            built[core]["vp1"] = _build_csr(R_pad, *pl["vp1"])
            built[core]["vc1"] = _build_csr(R_pad, *pl["vc1"])
            built[core]["l2"] = _build_csr(NSLOT, *pl["e2"])
            built[core]["vp2"] = _build_csr(NSLOT, *pl["vp2"])
            built[core]["vc2"] = _build_csr(NSLOT, *pl["vc2"])

        lay = {}
        for name, T in [("l1", T1), ("vp1", T1), ("vc1", T1),
                        ("l2", NSLOT // P), ("vp2", NSLOT // P),
                        ("vc2", NSLOT // P)]:
            layout, lpos = _csr_union_layout(
                [built[c][name][0] for c in range(NCORES)], T)
            lay[name] = (layout, lpos, len(layout))
        meta[f"lay_{dl}"] = {k: (v[0], v[2]) for k, v in lay.items()}

        for core in range(NCORES):
            pl = plans[(dl, core)]
            im = in_maps[core]
            for name, table in [("l1", emb), ("vp1", pri), ("vc1", cat),
                                ("vp2", pri), ("vc2", cat)]:
                layout, lpos, ntl = lay[name]
                IDX, VAL = _pack(*built[core][name], lpos, ntl)
                msg = table[IDX].astype(ml_dtypes.bfloat16)
                im[f"{name}_msg_{dl}"] = np.ascontiguousarray(
                    msg.reshape(ntl, P, D).transpose(1, 0, 2))
                im[f"{name}_val_{dl}"] = np.ascontiguousarray(
                    VAL.reshape(ntl, P).T.astype(ml_dtypes.bfloat16))
            layout, lpos, ntl = lay["l2"]
            IDX, VAL = _pack(*built[core]["l2"], lpos, ntl)
            assert IDX.max(initial=0) < 32768
            im[f"l2_idx_{dl}"] = _wrap16(IDX)
            im[f"l2_val_{dl}"] = np.ascontiguousarray(
                VAL.reshape(ntl, P).T.astype(np.float32))
            e0R = np.zeros((R_pad, D), np.float32)
            e0R[:pl["R"]] = emb[pl["R_orig"]]
            im[f"e0RT_{dl}"] = np.ascontiguousarray(e0R.T)
            im[f"slot_idx_{dl}"] = _wrap16(pl["slot_pos"])
            us = user_samp[core * NSLOT:(core + 1) * NSLOT][pl["pi2"]]
            im[f"convUT_{dl}"] = np.ascontiguousarray(conv[us].T)
            im[f"user_{dl}"] = np.ascontiguousarray(
                np.asarray(inputs[userk], np.float32)
                .reshape(UA // P, P, D).transpose(1, 0, 2))
        meta[f"pi2_{dl}"] = {c: plans[(dl, c)]["pi2"] for c in range(NCORES)}

    for core in range(NCORES):
        im = in_maps[core]
        im["W1"] = np.ascontiguousarray(
            np.asarray(inputs["wv1_W"], np.float32).reshape(3, D, D)
            .transpose(1, 0, 2))
        im["W2"] = np.asarray(inputs["wv2_W"], np.float32)
        im["W3"] = np.asarray(inputs["wv3_W"], np.float32)
        im["b1"] = np.asarray(inputs["wv1_b"], np.float32).reshape(D, 1)
        im["b2"] = np.asarray(inputs["wv2_b"], np.float32).reshape(D, 1)
        im["b3"] = np.asarray(inputs["wv3_b"], np.float32).reshape(D, 1)
        im["linW_a"] = np.asarray(inputs["lin_a_W"][1], np.float32)
        im["linb_a"] = np.asarray(inputs["lin_a_b"][1], np.float32).reshape(D, 1)
        im["linW_b"] = np.asarray(inputs["lin_b_W"][1], np.float32)
        im["linb_b"] = np.asarray(inputs["lin_b_b"][1], np.float32).reshape(D, 1)
    return meta, in_maps


# ------------------------------------------------------------- device kernel

KCHUNK = 8          # k-columns gathered/multiplied per block


def build_kernel(meta):
    import concourse.bacc as bacc
    import concourse.bass as bass
    import concourse.mybir as mybir
    from concourse.tile import TileContext
    from concourse.masks import make_identity

    BF = mybir.dt.bfloat16
    F32 = mybir.dt.float32
    R_pad = meta["R_pad"]
    T1 = R_pad // P
    NSLOT = meta["NSLOT"]
    UA = meta["UA"]
    TS = NSLOT // P

    nc = bacc.Bacc("TRN2", target_bir_lowering=False, debug=False,
                   num_devices=NCORES, num_swdge_queues=4)

    dram = {}

    def din(name, shape, dt):
        dram[name] = nc.dram_tensor(name, shape, dt, kind="ExternalInput")

    for dl in ("a", "b"):
        lay = meta[f"lay_{dl}"]
        for name in ("l1", "vp1", "vc1", "vp2", "vc2"):
            ntl = lay[name][1]
            din(f"{name}_msg_{dl}", [P, ntl, D], BF)
            din(f"{name}_val_{dl}", [P, ntl], BF)
        ntl2 = lay["l2"][1]
        din(f"l2_idx_{dl}", [P, (ntl2 * P) // 16], mybir.dt.int16)
        din(f"l2_val_{dl}", [P, ntl2], F32)
        din(f"e0RT_{dl}", [D, R_pad], F32)
        din(f"slot_idx_{dl}", [P, NSLOT // 16], mybir.dt.int16)
        din(f"convUT_{dl}", [UA, NSLOT], F32)
        din(f"user_{dl}", [P, UA // P, D], F32)
    for w, shape in [("W1", [D, 3, D]), ("W2", [D, D]), ("W3", [D, D]),
                     ("b1", [D, 1]), ("b2", [D, 1]), ("b3", [D, 1]),
                     ("linW_a", [D, D]), ("linb_a", [D, 1]),
                     ("linW_b", [D, D]), ("linb_b", [D, 1])]:
        din(w, shape, F32)
    out = nc.dram_tensor("scores", [2, NSLOT], F32, kind="ExternalOutput")
    scr = {dl: nc.dram_tensor(f"scr_{dl}", [R_pad, P], F32) for dl in ("a", "b")}

    AL = mybir.AluOpType
    AF = mybir.ActivationFunctionType

    def tile_ranges(layout):
        rng = {}
        for j, (t, k) in enumerate(layout):
            rng.setdefault(t, [j, j])
            rng[t][1] = j + 1
        return rng

    with TileContext(nc) as tc:
        with (
            tc.tile_pool(name="const", bufs=1) as cpool,
            tc.tile_pool(name="big", bufs=1) as bigp,
            tc.tile_pool(name="msg", bufs=3) as msgp,
            tc.tile_pool(name="mT", bufs=3) as mTp,
            tc.tile_pool(name="dense", bufs=3) as dnp,
            tc.tile_pool(name="stage", bufs=3) as stp,
            tc.tile_pool(name="conv", bufs=3) as convp,
            tc.tile_pool(name="gat", bufs=3) as gatp,
            tc.tile_pool(name="ps", bufs=5, space="PSUM") as psA,
            tc.tile_pool(name="psT", bufs=2, space="PSUM") as psT,
        ):
            identf = cpool.tile([P, P], F32)
            make_identity(nc, identf[:])
            ident_bf = cpool.tile([P, P], BF)
            nc.vector.tensor_copy(ident_bf[:], identf[:])
            ones64 = cpool.tile([D, 1], F32)
            nc.vector.memset(ones64[:], 1.0)

            W = {}
            for w in ("W1", "W2", "W3", "linW_a", "linW_b"):
                W[w] = cpool.tile(list(dram[w].shape), F32, tag=w, name=w)
                nc.sync.dma_start(W[w][:], dram[w].ap())
            Bv = {}
            for w in ("b1", "b2", "b3", "linb_a", "linb_b"):
                Bv[w] = cpool.tile([D, 1], F32, tag=w, name=w)
                nc.sync.dma_start(Bv[w][:], dram[w].ap())

            score_a = cpool.tile([1, NSLOT], F32, name="score_a")
            score_b = cpool.tile([1, NSLOT], F32, name="score_b")
            score_sb = {0: score_a, 1: score_b}

            for di, dl in enumerate(("a", "b")):
                linW = W[f"linW_{dl}"]
                linb = Bv[f"linb_{dl}"]
                lay = meta[f"lay_{dl}"]

                # ---------------- user side
                user_sb = bigp.tile([P, UA // P, D], F32, tag="user_sb")
                nc.sync.dma_start(user_sb[:], dram[f"user_{dl}"].ap())
                ps_u = psA.tile([D, NSLOT], F32, space="PSUM", tag="ps")
                for kt in range(UA // P):
                    cv = convp.tile([P, NSLOT], F32, tag="cv")
                    nc.sync.dma_start(
                        cv[:], dram[f"convUT_{dl}"].ap()[kt * P:(kt + 1) * P, :])
                    nc.tensor.matmul(ps_u[:], lhsT=user_sb[:, kt, :], rhs=cv[:],
                                     start=(kt == 0), stop=(kt == UA // P - 1))
                Mu1T = bigp.tile([D, NSLOT], F32, tag="Mu1T")
                nc.scalar.copy(Mu1T[:], ps_u[:])
                ps_z = psA.tile([D, NSLOT], F32, space="PSUM", tag="ps")
                nc.tensor.matmul(ps_z[:], lhsT=linW[:], rhs=Mu1T[:],
                                 start=True, stop=True)
                uaT = bigp.tile([D, NSLOT], F32, tag="uaT")
                nc.scalar.activation(uaT[:], ps_z[:], AF.Relu,
                                     bias=linb[:], scale=1.0)
                nc.vector.tensor_add(uaT[:], uaT[:], Mu1T[:])

                # ---------------- generic chunked spmm accumulate
                def spmm_acc(name, j0, j1, ps, vdt, first):
                    """Accumulate msg tiles [j0, j1) of shard `name` into psum
                    ps [64, 128]. Returns whether anything was accumulated."""
                    idt = ident_bf if vdt == BF else identf
                    for c0 in range(j0, j1, KCHUNK):
                        c1 = min(c0 + KCHUNK, j1)
                        nb = c1 - c0
                        m = msgp.tile([P, KCHUNK, D], vdt, tag=f"m_{name}")
                        nc.sync.dma_start(
                            m[:, 0:nb, :],
                            dram[f"{name}_msg_{dl}"].ap()[:, c0:c1, :])
                        v = msgp.tile([P, KCHUNK], vdt, tag=f"v_{name}")
                        nc.sync.dma_start(
                            v[:, 0:nb], dram[f"{name}_val_{dl}"].ap()[:, c0:c1])
                        sm = msgp.tile([P, KCHUNK, D], vdt, tag=f"s_{name}")
                        nc.vector.tensor_tensor(
                            out=sm[:, 0:nb, :], in0=m[:, 0:nb, :],
                            in1=v[:, 0:nb].to_broadcast([P, nb, D]), op=AL.mult)
                        for j in range(nb):
                            nc.tensor.matmul(
                                ps[:], lhsT=sm[:, j, :], rhs=idt[:],
                                start=first and (c0 == j0 and j == 0),
                                stop=(c1 == j1 and j == nb - 1))
                    return j1 > j0

                r_l1 = tile_ranges(lay["l1"][0])
                r_vp = tile_ranges(lay["vp1"][0])
                r_vc = tile_ranges(lay["vc1"][0])

                def spmm_tile(name, rng, t):
                    res = mTp.tile([D, P], F32, tag=f"r_{name}")
                    if t in rng:
                        ps = psA.tile([D, P], F32, space="PSUM", tag="ps")
                        spmm_acc(name, rng[t][0], rng[t][1], ps, BF, True)
                        nc.scalar.copy(res[:], ps[:])
                    else:
                        nc.vector.memset(res[:], 0.0)
                    return res

                # ---------------- item layer 1, per dest tile
                for t in range(T1):
                    e0c = mTp.tile([D, P], F32, tag="e0c")
                    nc.sync.dma_start(
                        e0c[:], dram[f"e0RT_{dl}"].ap()[:, t * P:(t + 1) * P])
                    madj = spmm_tile("l1", r_l1, t)
                    mp = spmm_tile("vp1", r_vp, t)
                    mc = spmm_tile("vc1", r_vc, t)

                    ps_b = psA.tile([D, P], F32, space="PSUM", tag="ps")
                    nc.tensor.matmul(ps_b[:], lhsT=W["W1"][:, 0, :], rhs=e0c[:],
                                     start=True, stop=False)
                    nc.tensor.matmul(ps_b[:], lhsT=W["W1"][:, 1, :], rhs=mp[:],
                                     start=False, stop=False)
                    nc.tensor.matmul(ps_b[:], lhsT=W["W1"][:, 2, :], rhs=mc[:],
                                     start=False, stop=True)
                    base = dnp.tile([D, P], F32, tag="base")
                    nc.scalar.activation(base[:], ps_b[:], AF.Identity,
                                         bias=Bv["b1"][:], scale=1.0)
                    ps_g = psA.tile([D, P], F32, space="PSUM", tag="ps")
                    nc.tensor.matmul(ps_g[:], lhsT=W["W2"][:], rhs=mp[:],
                                     start=True, stop=True)
                    g1 = dnp.tile([D, P], F32, tag="g1")
                    nc.vector.tensor_add(g1[:], ps_g[:], base[:])
                    nc.scalar.activation(g1[:], g1[:], AF.Tanh,
                                         bias=Bv["b2"][:], scale=1.0)
                    ps_h = psA.tile([D, P], F32, space="PSUM", tag="ps")
                    nc.tensor.matmul(ps_h[:], lhsT=W["W3"][:], rhs=mc[:],
                                     start=True, stop=True)
                    g2 = dnp.tile([D, P], F32, tag="g2")
                    nc.vector.tensor_add(g2[:], ps_h[:], base[:])
                    nc.scalar.activation(g2[:], g2[:], AF.Tanh,
                                         bias=Bv["b3"][:], scale=1.0)

                    it1 = dnp.tile([D, P], F32, tag="it1")
                    nc.vector.tensor_tensor(out=it1[:], in0=g1[:], in1=mp[:],
                                            op=AL.mult)
                    tmp = dnp.tile([D, P], F32, tag="tmp")
                    nc.vector.tensor_tensor(out=tmp[:], in0=g2[:], in1=mc[:],
                                            op=AL.mult)
                    nc.vector.tensor_add(it1[:], it1[:], tmp[:])
                    nc.vector.tensor_add(it1[:], it1[:], madj[:])
                    nc.vector.tensor_add(it1[:], it1[:], e0c[:])

                    ps_w = psA.tile([D, P], F32, space="PSUM", tag="ps")
                    nc.tensor.matmul(ps_w[:], lhsT=linW[:], rhs=it1[:],
                                     start=True, stop=True)
                    stg = stp.tile([P, P], F32, tag="stg")
                    nc.scalar.activation(stg[0:D, :], ps_w[:], AF.Relu,
                                         bias=linb[:], scale=1.0)
                    nc.vector.tensor_add(stg[0:D, :], stg[0:D, :], it1[:])
                    nc.vector.tensor_add(stg[D:P, :], it1[:], e0c[:])
                    ps_t = psT.tile([P, P], F32, space="PSUM", tag="pst")
                    nc.tensor.transpose(ps_t[:], stg[:], identf[:])
                    rowt = stp.tile([P, P], F32, tag="rowt")
                    nc.scalar.copy(rowt[:], ps_t[:])
                    nc.sync.dma_start(scr[dl].ap()[t * P:(t + 1) * P, :], rowt[:])

                # ---------------- layer 2 (slot space)
                ntl2 = lay["l2"][1]
                r_l2 = tile_ranges(lay["l2"][0])
                r_vp2 = tile_ranges(lay["vp2"][0])
                r_vc2 = tile_ranges(lay["vc2"][0])
                l2_idx = bigp.tile([P, (ntl2 * P) // 16], mybir.dt.int16,
                                   tag="l2_idx")
                nc.sync.dma_start(l2_idx[:], dram[f"l2_idx_{dl}"].ap())
                l2_val = bigp.tile([P, ntl2], F32, tag="l2_val")
                nc.sync.dma_start(l2_val[:], dram[f"l2_val_{dl}"].ap())
                slot_idx = bigp.tile([P, NSLOT // 16], mybir.dt.int16,
                                     tag="slot_idx")
                nc.sync.dma_start(slot_idx[:], dram[f"slot_idx_{dl}"].ap())

                e1s = bigp.tile([D, NSLOT], F32, tag="e1s")
                qs = bigp.tile([D, NSLOT], F32, tag="qs")
                for pi, (dst, off) in enumerate(((e1s, 0), (qs, D))):
                    g = gatp.tile([P, TS, D], F32, tag="slotg")
                    nc.gpsimd.dma_gather(
                        out_ap=g[:], in_ap=scr[dl].ap()[:, off:off + D],
                        idxs_ap=slot_idx[:], num_idxs=NSLOT, num_idxs_reg=NSLOT,
                        elem_size=D, elem_step=P, single_packet=False,
                        queue_num=pi)
                    for j in range(TS):
                        ps_f = psA.tile([D, P], F32, space="PSUM", tag="ps")
                        nc.tensor.matmul(ps_f[:], lhsT=g[:, j, :], rhs=identf[:],
                                         start=True, stop=True)
                        nc.scalar.copy(dst[:, j * P:(j + 1) * P], ps_f[:])

                m2 = bigp.tile([D, NSLOT], F32, tag="m2")
                for t in range(TS):
                    if t in r_l2:
                        j0, j1 = r_l2[t]
                        ps = psA.tile([D, P], F32, space="PSUM", tag="ps")
                        for c0 in range(j0, j1, KCHUNK):
                            c1 = min(c0 + KCHUNK, j1)
                            nb = c1 - c0
                            g = gatp.tile([P, KCHUNK, D], F32, tag="l2g")
                            nc.gpsimd.dma_gather(
                                out_ap=g[:, 0:nb, :], in_ap=scr[dl].ap()[:, 0:D],
                                idxs_ap=l2_idx[:, c0 * 8:c1 * 8],
                                num_idxs=nb * P, num_idxs_reg=nb * P,
                                elem_size=D, elem_step=P, single_packet=False,
                                queue_num=(c0 // KCHUNK) % 4)
                            sg = gatp.tile([P, KCHUNK, D], F32, tag="l2sg")
                            nc.vector.tensor_tensor(
                                out=sg[:, 0:nb, :], in0=g[:, 0:nb, :],
                                in1=l2_val[:, c0:c1].to_broadcast([P, nb, D]),
                                op=AL.mult)
                            for j in range(nb):
                                nc.tensor.matmul(
                                    ps[:], lhsT=sg[:, j, :], rhs=identf[:],
                                    start=(c0 == j0 and j == 0),
                                    stop=(c1 == j1 and j == nb - 1))
                        nc.scalar.copy(m2[:, t * P:(t + 1) * P], ps[:])
                    else:
                        nc.vector.memset(m2[:, t * P:(t + 1) * P], 0.0)

                def spmm_slot(name, rngd, tagn):
                    res = bigp.tile([D, NSLOT], F32, tag=tagn)
                    for t in range(TS):
                        if t in rngd:
                            ps = psA.tile([D, P], F32, space="PSUM", tag="ps")
                            spmm_acc(name, rngd[t][0], rngd[t][1], ps, BF, True)
                            nc.scalar.copy(res[:, t * P:(t + 1) * P], ps[:])
                        else:
                            nc.vector.memset(res[:, t * P:(t + 1) * P], 0.0)
                    return res

                mp2 = spmm_slot("vp2", r_vp2, "mp2")
                mc2 = spmm_slot("vc2", r_vc2, "mc2")

                ps_b2 = psA.tile([D, NSLOT], F32, space="PSUM", tag="ps")
                nc.tensor.matmul(ps_b2[:], lhsT=W["W1"][:, 0, :], rhs=e1s[:],
                                 start=True, stop=False)
                nc.tensor.matmul(ps_b2[:], lhsT=W["W1"][:, 1, :], rhs=mp2[:],
                                 start=False, stop=False)
                nc.tensor.matmul(ps_b2[:], lhsT=W["W1"][:, 2, :], rhs=mc2[:],
                                 start=False, stop=True)
                base2 = bigp.tile([D, NSLOT], F32, tag="base2")
                nc.scalar.activation(base2[:], ps_b2[:], AF.Identity,
                                     bias=Bv["b1"][:], scale=1.0)
                ps_i = psA.tile([D, NSLOT], F32, space="PSUM", tag="ps")
                nc.tensor.matmul(ps_i[:], lhsT=W["W2"][:], rhs=mp2[:],
                                 start=True, stop=True)
                g1b = bigp.tile([D, NSLOT], F32, tag="g1b")
                nc.vector.tensor_add(g1b[:], ps_i[:], base2[:])
                nc.scalar.activation(g1b[:], g1b[:], AF.Tanh,
                                     bias=Bv["b2"][:], scale=1.0)
                ps_j = psA.tile([D, NSLOT], F32, space="PSUM", tag="ps")
                nc.tensor.matmul(ps_j[:], lhsT=W["W3"][:], rhs=mc2[:],
                                 start=True, stop=True)
                g2b = bigp.tile([D, NSLOT], F32, tag="g2b")
                nc.vector.tensor_add(g2b[:], ps_j[:], base2[:])
                nc.scalar.activation(g2b[:], g2b[:], AF.Tanh,
                                     bias=Bv["b3"][:], scale=1.0)

                ia3 = bigp.tile([D, NSLOT], F32, tag="ia3")
                nc.vector.tensor_tensor(out=ia3[:], in0=g1b[:], in1=mp2[:],
                                        op=AL.mult)
                tt = bigp.tile([D, NSLOT], F32, tag="tt")
                nc.vector.tensor_tensor(out=tt[:], in0=g2b[:], in1=mc2[:],
                                        op=AL.mult)
                nc.vector.tensor_add(ia3[:], ia3[:], tt[:])
                nc.vector.tensor_add(ia3[:], ia3[:], m2[:])
                nc.vector.tensor_add(ia3[:], ia3[:], e1s[:])
                nc.vector.tensor_add(ia3[:], ia3[:], qs[:])
                nc.vector.tensor_tensor(out=ia3[:], in0=ia3[:], in1=uaT[:],
                                        op=AL.mult)
                ps_s = psA.tile([1, NSLOT], F32, space="PSUM", tag="ps")
                nc.tensor.matmul(ps_s[:], lhsT=ones64[:], rhs=ia3[:],
                                 start=True, stop=True)
                nc.scalar.activation(score_sb[di][:], ps_s[:], AF.Copy,
                                     scale=1.0 / 3.0)

            nc.sync.dma_start(out.ap()[0:1, :], score_a[:])
            nc.sync.dma_start(out.ap()[1:2, :], score_b[:])
    nc.compile()
    return nc


# ------------------------------------------------------------------ frontend

def kernel(**inputs):
    from concourse.bass_utils import run_bass_kernel_spmd
    meta, in_maps = prepare_inputs(inputs)
    nc = build_kernel(meta)
    res = run_bass_kernel_spmd(nc, in_maps, core_ids=list(range(NCORES)))
    NSLOT = meta["NSLOT"]
    scores = np.zeros((meta["BATCH"], 2), np.float32)
    for core in range(NCORES):
        sc = res.results[core]["scores"]
        for di, dl in enumerate(("a", "b")):
            pi2 = meta[f"pi2_{dl}"][core]
            s = np.zeros(NSLOT, np.float32)
            s[pi2] = sc[di]
            scores[core * NSLOT:(core + 1) * NSLOT, di] = s
    return scores

